# revision 44
# baseline (speedup 1.0000x reference)
"""Causal multi-head attention on 8 trn2 NeuronCores.

Problem: B=2, T=2048, C=1024, H=16 heads, D=64, fp32 reference.
    q/k/v = x @ W{q,k,v}.T ; causal softmax(q k^T / sqrt(D)) @ v ; out @ Wo.T

Sharding (Megatron-style): data-parallel over batch (2 groups of 4 cores),
tensor-parallel over heads within a group (4 heads per core; Wq/Wk/Wv
column-sharded, Wo row-sharded). Each core emits a partial y[b].T in fp16;
the host sums the 4 partials per batch in f32 and transposes back.

Per-core device program (all matmul inputs fp16; PSUM accumulates f32).

The PE clock p-states on trn2 halve matmul speed unless the PE has been
continuously busy for 3 us (2.4 GHz after 3 us, 1.2 GHz before, 0.65 GHz
cold).  The v1 phase-sequential program (projections, then per-head
attention, then output proj) left the PE with a 300-1100 ns stall every
few exp-bounded iterations, so phase 2 never ramped and ran entirely at
1.2 GHz (measured 427 ns per 512-col matmul).  This version emits ONE
interleaved PE instruction stream:

  - projection chains (q/k/v, [128,512] PSUM tiles) and output-projection
    chains are interleaved BETWEEN attention iterations, one chain per
    iteration slot, so the PE always has independent work while ACT runs
    exp and never idles -> stays at 2.4 GHz;
  - the softmax denominator is produced by the AV matmul itself on PSUM
    partitions 64:128 (the v tile per head is [128,128]: cols 0:64 =
    pad-scaled v, cols 64:128 = pad broadcast 64-wide).  The out
    partitions were idle anyway (M=65 before), so the replication is free
    PE-wise and kills the gpsimd partition_broadcast + single-lane
    reciprocal chain of v1: normalization is now copy[64,512] (PSUM->SBUF)
    -> reciprocal_approx_fast -> mul, all short multi-lane DVE/Pool ops;
  - DMAs are issued only from the sync/vector/gpsimd sequencers (ACT
    stays free for exp); inputs are sliced in first-use order so the
    first projection chain starts ~4 us after the preamble;
  - all PSUM lives in one pool: proj [128,512]x2 (2 banks) + scores
    [128,1024]x2 (4 banks) + ctx [128,1024]x1 (2 banks) = 8 banks.

Schedule (one PE stream; slots = proj/output chains between iterations):
  pre:   q(jc0,n0) k(jc0,n0) q(jc0,n1) k(jc0,n1)
  h0-A:  v01 v23 v45 v67 q(jc0,n2) k(jc0,n2) q(jc0,n3) k(jc0,n3)
  h0-B:  v89 .. v14-15, q(jc1,n0) k(jc1,n0) q(jc1,n1) k(jc1,n1)
  h1-A:  q(jc1,n2) k(jc1,n2) q(jc1,n3) k(jc1,n3)
  h1-B .. h3-A: (no slots left; bare attention pipeline)
  h3-B:  output-projection tg=0 units (cols 0:1024, all heads final)
  post:  output-projection tg=1 units + final DMAs
"""

import os

import numpy as np

B, T, C, H, D = 2, 2048, 1024, 16, 64
NCORES = 8
GROUPS = 4          # tensor-parallel groups per batch
HPC = H // GROUPS   # heads per core = 4
J = HPC * D         # per-core projection width = 256
P = 128
NT = T // P         # 16 key chunks
KC = C // P         # 8 contraction chunks
NQ = T // 512       # 4 query 512-blocks
E2 = 2 * D          # 128: head dim + 64 denominator-replica columns

MM_DTYPE = os.environ.get("MM_DTYPE", "float16")  # "float16" or "bfloat16"
MASK_ENG = os.environ.get("MASK_ENG", "dve")  # "pool" | "dve"
DRAIN_MODE = os.environ.get("DRAIN_MODE", "alt")  # "alt" | "dve"
_COMPILED = None


def build_program(dtype_mm=None, variant="full", mult=1):
    """Emit the SPMD bass program (same on all 8 cores).

    variant: "full" | "qkv" (projections only) | "attn" (no output proj)
    """
    import concourse.bass as bass
    import concourse.mybir as mybir
    import concourse.tile as tile
    from concourse import bacc
    from concourse.masks import make_upper_triangular

    dtype_mm = dtype_mm or MM_DTYPE
    f32 = mybir.dt.float32
    md = getattr(mybir.dt, dtype_mm)

    nc = bacc.Bacc("TRN2", target_bir_lowering=False, debug=False)

    xT = nc.dram_tensor("xT", [C, T], md, kind="ExternalInput").ap()
    wq = nc.dram_tensor("wq_t", [C, J], md, kind="ExternalInput").ap()
    wk = nc.dram_tensor("wk_t", [C, J], md, kind="ExternalInput").ap()
    wv = nc.dram_tensor("wv_t", [C, J], md, kind="ExternalInput").ap()
    wo = nc.dram_tensor("wo_t", [J, C], md, kind="ExternalInput").ap()
    pad = nc.dram_tensor("pad", [T, 1], f32, kind="ExternalInput").ap()
    yT = nc.dram_tensor("yT", [C, T], md, kind="ExternalOutput").ap()

    def _round_up_size(size):
        for v in (32, 64, 128):
            if v >= size:
                return v

    def mm_noload(out, lhsT, rhs, start, stop):
        """Matmul that reuses the PE array's already-loaded stationary operand.

        The immediately preceding PE matmul MUST have loaded the identical
        lhsT."""
        te = nc.tensor
        ifmap_ap = te.lower_ap(rhs.opt({0}), opt=False)
        weights_ap = te.lower_ap(lhsT.opt({0}), opt=False, for_matmul_weights=True)
        out_ap = te.lower_ap(out)
        return te.add_instruction(
            mybir.InstMatmult(
                name=nc.get_next_instruction_name(),
                replication_resolution=0,
                replication_shift_amnt=0,
                replication_num_rows=0,
                start_tensor_calc=start,
                stop_tensor_calc=stop,
                ins=[ifmap_ap, weights_ap],
                outs=[out_ap],
                perf_mode=None,
                is_transpose=None,
                ifmap_quant_offset=None,
                weights_quant_offset=None,
                bass_skip_group_check=False,
                tile_position=(lhsT.base_partition(), out.base_partition()),
                tile_size=(
                    _round_up_size(rhs.partition_size()),
                    _round_up_size(out.partition_size()),
                ),
                ldweights=False,
            )
        )

    def dump_debug(src_sb):
        """Debug variants: dump a [128, 2, T] tile to yT so output deps exist."""
        for jc in range(2):
            for tn in range(NQ):
                nc.sync.dma_start(
                    out=yT[jc * P : (jc + 1) * P, tn * 512 : (tn + 1) * 512],
                    in_=src_sb[:, jc, tn * 512 : (tn + 1) * 512],
                )

    with tile.TileContext(nc) as tc:
        with (
            tc.tile_pool(name="const", bufs=1) as const_pool,
            tc.tile_pool(name="sb", bufs=1) as sb_pool,
            tc.tile_pool(name="expp", bufs=4) as exp_pool,
            tc.tile_pool(name="norm", bufs=2) as norm_pool,
            tc.tile_pool(name="ysb", bufs=4) as y_pool,
            tc.tile_pool(name="ps", bufs=1, space="PSUM") as ps_pool,
        ):
            # mask[tk, tq] = 1.0 iff tk <= tq (causal keep-region); applied as
            # a DVE multiply on the diagonal 128-block after exp
            mask = const_pool.tile([P, P], md)
            pad_sb = const_pool.tile([P, NT], f32)

            qT_sb = sb_pool.tile([P, 2, T], md)  # [128, jc, t] j=jc*128+p
            kT_sb = sb_pool.tile([P, 2, T], md)
            v1_sb = sb_pool.tile([P, NT, HPC * E2], md)
            wo_sb = sb_pool.tile([P, 2, C], md)
            ctxn_sb = sb_pool.tile([P, 2, T], md)  # normalized ctxT, [j, t]
            w_tiles = {
                name: sb_pool.tile([P, KC, J], md, tag=f"w{name}", name=f"w{name}_sb")
                for name in ("q", "k", "v")
            }
            xT_sb = sb_pool.tile([P, KC, T], md)

            # ---- input DMAs: first-use order, striped over the sync /
            # vector / gpsimd DGE queues (ACT's sequencer stays free for
            # exp).  First q chain needs wq + x0 only. ----
            xv = xT.rearrange("(kc p) t -> p kc t", p=P)

            def dma_w(eng, name, w_ap, k0=0, k1=KC):
                wv_ = w_ap.rearrange("(kc p) j -> p kc j", p=P)
                eng.dma_start(out=w_tiles[name][:, k0:k1, :], in_=wv_[:, k0:k1, :])

            def dma_x(eng, n, k0=0, k1=KC):
                cs = slice(n * 512, (n + 1) * 512)
                eng.dma_start(out=xT_sb[:, k0:k1, cs], in_=xv[:, k0:k1, cs])

            pad_v = pad.rearrange("(i p) one -> p (i one)", p=P)
            nc.gpsimd.dma_start(out=pad_sb, in_=pad_v)
            # Global first-use order, every tensor striped across all three
            # queues so each tensor's arrival time ~= its position in the
            # global byte order (the 16 HW DMA engines drain the queues
            # round-robin).  The input stream is HBM-bandwidth-bound
            # (~280 GB/s aggregate measured), so arrival order ==
            # consumption order is what minimizes early PE stalls.  The
            # scalar (ACT) queue gets only three early gens so its
            # sequencer is long done before the exp stream starts.
            dma_w(nc.sync, "q", wq, 0, 4)
            dma_x(nc.scalar, 0, 0, 4)
            dma_w(nc.gpsimd, "q", wq, 4, 8)
            dma_x(nc.sync, 0, 4, 6)
            dma_x(nc.gpsimd, 0, 6, 8)
            dma_w(nc.sync, "k", wk, 0, 4)
            dma_w(nc.scalar, "k", wk, 4, 8)
            dma_x(nc.sync, 1, 0, 3)
            dma_x(nc.scalar, 1, 3, 5)
            dma_x(nc.gpsimd, 1, 5, 8)
            dma_w(nc.sync, "v", wv, 0, 4)
            dma_w(nc.gpsimd, "v", wv, 4, 8)
            dma_x(nc.sync, 2, 0, 4)
            dma_x(nc.gpsimd, 2, 4, 8)
            dma_x(nc.sync, 3, 0, 4)
            dma_x(nc.gpsimd, 3, 4, 8)
            wo_v = wo.rearrange("(jc p) c -> p jc c", p=P)
            nc.gpsimd.dma_start(out=wo_sb[:, 1, :], in_=wo_v[:, 1, :])
            nc.sync.dma_start(out=wo_sb[:, 0, :], in_=wo_v[:, 0, :])
            make_upper_triangular(nc, mask, val=1.0, diag=True)

            # denominator-replica columns of v1: cols 64:128 of each head's
            # [128,128] block = pad value, so the AV matmul accumulates the
            # softmax denominator on ctx partitions 64:128.  Two strided
            # copies (DVE/Pool halves) off the critical path.
            ones_all = v1_sb.rearrange("p i (h e) -> p i h e", e=E2)[:, :, :, D:E2]
            def pad_bcast(i0, i1):
                return bass.AP(
                    tensor=pad_sb.tensor,
                    offset=pad_sb.offset + i0 * pad_sb.ap[1][0],
                    ap=[pad_sb.ap[0], [pad_sb.ap[1][0], i1 - i0], [0, HPC], [0, D]],
                )
            # SBUF->SBUF, but NOT on gpsimd: its strided-copy rate is ~7x
            # slower than DVE (measured 8 us vs 1.2 us for this copy)
            nc.vector.tensor_copy(ones_all[:, 0:8], pad_bcast(0, 8))
            if os.environ.get("ONES_ENG", "act") == "act":
                nc.scalar.copy(ones_all[:, 8:NT], pad_bcast(8, NT))
            else:
                nc.vector.tensor_copy(ones_all[:, 8:NT], pad_bcast(8, NT))

            # ---- PE work units (each ~0.4-1.8 us of PE work) ----
            # PSUM is only reachable from DVE and ACT; ACT is the exp engine,
            # so projection drains go to DVE and output drains alternate
            # DVE/ACT (they run in the tail where exp is draining out).
            drain_flip = [0]

            def drain_eng():
                if DRAIN_MODE == "dve":
                    return nc.vector.tensor_copy
                drain_flip[0] ^= 1
                return nc.vector.tensor_copy if drain_flip[0] else nc.scalar.copy

            # unit psum tiles share the "s" tag ring with the scores tiles
            # ([128,512] fits in a [128,1024] slot): the ring is bufs=3, so
            # wherever units are interleaved they occupy ring slots, and in
            # bare attention stretches the scores get full triple-buffering
            # (scores(i+3) <- exp(i), never a double-buffer stall).
            def unit_qk(name, jc, n):
                def emit():
                    cs = slice(n * 512, (n + 1) * 512)
                    dst = qT_sb if name == "q" else kT_sb
                    ps = ps_pool.tile([P, 512], f32, tag="s", name="proj_ps", bufs=3)
                    for kc in range(KC):
                        nc.tensor.matmul(
                            ps,
                            lhsT=w_tiles[name][:, kc, jc * P : (jc + 1) * P],
                            rhs=xT_sb[:, kc, cs],
                            start=(kc == 0),
                            stop=(kc == KC - 1),
                        )
                    drain_eng()(dst[:, jc, cs], ps)
                return emit

            def unit_v(i0):
                # two t-chunks (i0, i0+1) in one [128,512] psum tile
                def emit():
                    ps = ps_pool.tile([P, 512], f32, tag="s", name="v_ps", bufs=3)
                    for ii in range(2):
                        i = i0 + ii
                        for kc in range(KC):
                            nc.tensor.matmul(
                                ps[:, ii * J : (ii + 1) * J],
                                lhsT=xT_sb[:, kc, i * P : (i + 1) * P],
                                rhs=w_tiles["v"][:, kc, :],
                                start=(kc == 0),
                                stop=(kc == KC - 1),
                            )
                    for ii in range(2):
                        i = i0 + ii
                        v1_v = v1_sb[:, i, :].rearrange("p (h e) -> p h e", e=E2)[
                            :, :, 0:D
                        ]
                        ps_v = ps[:, ii * J : (ii + 1) * J].rearrange(
                            "p (h d) -> p h d", d=D
                        )
                        if DRAIN_MODE != "dve":
                            drain_flip[0] ^= 1
                        if drain_flip[0] or DRAIN_MODE == "dve":
                            nc.vector.tensor_scalar_mul(
                                v1_v, ps_v, pad_sb[:, i : i + 1]
                            )
                        else:
                            nc.scalar.mul(v1_v, ps_v, pad_sb[:, i : i + 1])
                return emit

            dma_rot = [0]

            def unit_y(oc, tn, post=False):
                # output proj for row-block oc, cols [tn*512, tn*512+512).
                # Drain on DVE only (ACT still runs the last exps during the
                # h3-B slots); post-region DMAs also use the scalar queue
                # (ACT's sequencer is free there, HWDGE gen is 625 ns vs
                # gpsimd SWDGE's 1038 ns).
                def emit():
                    ps = ps_pool.tile([P, 512], f32, tag="s", name="y_ps", bufs=3)
                    for jcc in range(2):
                        wo_c = wo_sb[:, jcc, oc * P : (oc + 1) * P]
                        rhs = ctxn_sb[:, jcc, tn * 512 : (tn + 1) * 512]
                        nc.tensor.matmul(
                            ps, lhsT=wo_c, rhs=rhs,
                            start=(jcc == 0), stop=(jcc == 1),
                        )
                    yo = y_pool.tile([P, 512], md, tag="yo", name="yo")
                    # tn<=2 drains on ACT (it has exp slack in the h3-B slots
                    # and is idle in post); tn=3 on DVE, which by then has
                    # just finished the final normalization chain -- keeping
                    # that chain unqueued behind drains is what lets the tn3
                    # units start promptly
                    ym = os.environ.get("Y_DRAIN", "mix")
                    if ym == "alt" or (ym == "mix" and tn >= 2):
                        drain_eng()(yo, ps)
                    elif ym == "mix" or tn <= 2:
                        nc.scalar.copy(yo, ps)
                    else:
                        nc.vector.tensor_copy(yo, ps)
                    dma_rot[0] += 1
                    if post:
                        eng = (nc.sync, nc.scalar, nc.gpsimd)[dma_rot[0] % 3]
                    else:
                        eng = (nc.sync, nc.gpsimd)[dma_rot[0] % 2]
                    eng.dma_start(
                        out=yT[oc * P : (oc + 1) * P, tn * 512 : (tn + 1) * 512],
                        in_=yo,
                    )
                return emit

            # slot lists per pass (consumed one per attention iteration),
            # spread as late as dependencies allow so the interleaved region
            # covers as much of the attention stream as possible:
            #   q(jc,n2/n3) before that jc's pass B starts; k(jc,n2) before
            #   its pass-B i=8, k(jc,n3) before i=12; v(i) before AV(i)
            #   retires at i+2.
            _n = None
            pre_units = [
                unit_qk("q", 0, 0), unit_qk("k", 0, 0),
                unit_qk("q", 0, 1), unit_qk("k", 0, 1),
            ]
            slots = {
                (0, 0): [unit_v(0), unit_v(2), unit_v(4), unit_v(6),
                         unit_qk("q", 0, 2), unit_qk("q", 0, 3), _n, _n],
                (0, 1024): [unit_qk("k", 0, 2), _n, unit_v(8), _n,
                            unit_qk("k", 0, 3), _n, unit_v(10), _n,
                            unit_v(12), _n, unit_v(14), _n, _n, _n, _n, _n],
                (1, 0): [unit_qk("q", 1, 0), _n, _n, _n,
                         unit_qk("k", 1, 0), _n, _n, _n],
                (1, 1024): [unit_qk("q", 1, 1), _n, _n, unit_qk("k", 1, 1),
                            _n, _n, unit_qk("q", 1, 2), _n, _n,
                            unit_qk("q", 1, 3), _n, _n, _n, _n, _n, _n],
                (2, 0): [unit_qk("k", 1, 2), _n, _n, _n,
                         unit_qk("k", 1, 3), _n, _n, _n],
                # first two slots empty: the previous pass's deferred
                # normalization chain is only flushed at iterations 0-1, and
                # the output units read ctxn
                (3, 1024): [_n, _n]
                + [unit_y(oc, 0) for oc in range(KC)]
                + [unit_y(oc, 1) for oc in range(KC)],
            }
            # tn=2 units don't need the final quarter's normalization -> they
            # run while the tn=3 chain completes
            post_units = [unit_y(oc, 2, post=True) for oc in range(KC)] + [
                unit_y(oc, 3, post=True) for oc in range(KC)
            ]
            if variant == "qkv":
                for key in ((0, 0), (0, 1024), (1, 0), (1, 1024), (2, 0)):
                    pre_units.extend(u for u in slots[key] if u)
                for u in pre_units:
                    u()
                dump_debug(qT_sb)
            elif variant in ("attn", "full"):
              for _rep in range(mult):
                for u in pre_units:
                    u()

                # ---- attention: per head, two query-column passes ----
                carry = []  # deferred (emit_fn, due_i) into next pass
                for h in range(HPC):
                    jc, poff = h // 2, (h % 2) * D
                    qTh = qT_sb[poff : poff + D, jc, :]
                    kTh = kT_sb[poff : poff + D, jc, :]
                    for c0p, nt_pass in ((0, 8), (1024, NT)):
                        c1p = c0p + 1024
                        my_slots = slots.get((h, c0p), [])
                        ctx = ps_pool.tile(
                            [P, 1024], f32, tag="ctx", name=f"ctx_{h}_{c0p}", bufs=1
                        )
                        bc = norm_pool.tile([D, 1024], f32, tag="bc")

                        def emit_av(i, e, ctx=ctx, h=h, c0p=c0p, c1p=c1p):
                            v1h = v1_sb[:, i, h * E2 : (h + 1) * E2]
                            first = True
                            for n in range(max(i // 4, c0p // 512), c1p // 512):
                                c0 = max(i * P, n * 512)
                                dst = ctx[:, c0 - c0p : (n + 1) * 512 - c0p]
                                rhs = e[:, c0 - c0p : (n + 1) * 512 - c0p]
                                if first:
                                    nc.tensor.matmul(
                                        dst, lhsT=v1h, rhs=rhs,
                                        start=(i == 0), stop=(i == 4 * n + 3),
                                    )
                                    first = False
                                else:
                                    mm_noload(dst, v1h, rhs, i == 0, i == 4 * n + 3)

                        def emit_recip(n, ctx=ctx, bc=bc, c0p=c0p):
                            cols = slice(n * 512 - c0p, (n + 1) * 512 - c0p)
                            # denominator is on PSUM partitions 64:128; stage
                            # through SBUF (reciprocal_approx_fast reads
                            # garbage from PSUM on hw)
                            nc.vector.tensor_copy(bc[:, cols], ctx[D:E2, cols])
                            nc.vector.reciprocal_approx_fast(bc[:, cols], bc[:, cols])

                        def emit_mul(n, ctx=ctx, bc=bc, poff=poff, jc=jc, c0p=c0p):
                            cols = slice(n * 512 - c0p, (n + 1) * 512 - c0p)
                            nc.vector.tensor_mul(
                                ctxn_sb[poff : poff + D, jc, n * 512 : (n + 1) * 512],
                                ctx[0:D, cols],
                                bc[:, cols],
                            )

                        # software pipeline, lag 2: AV(i-2) is emitted BEFORE
                        # scores(i) so the in-order PE streams through it
                        # while exp(i-1) is still in flight
                        pend = []
                        final_pass = h == HPC - 1 and c0p == 1024

                        def retire_one(
                            cur_i, c0p=c0p, c1p=c1p, final_pass=final_pass
                        ):
                            ii, ee = pend.pop(0)
                            emit_av(ii, ee)
                            n = ii // 4
                            if ii % 4 == 3 and c0p // 512 <= n < c1p // 512:
                                if ii == c1p // P - 1 and not final_pass:
                                    # pass's last quarter: defer the WHOLE
                                    # chain into the next pass's first two
                                    # iterations, whose scores hide its
                                    # latency (emission stays before the next
                                    # pass's AV(0) ctx overwrite at i=2)
                                    carry.append(
                                        (lambda n=n, f=emit_recip: f(n), 0)
                                    )
                                    carry.append(
                                        (lambda n=n, f=emit_mul: f(n), 1)
                                    )
                                elif final_pass and ii == c1p // P - 1:
                                    emit_recip(n)
                                    emit_mul(n)
                                else:
                                    emit_recip(n)
                                    carry.append(
                                        (lambda n=n, f=emit_mul: f(n), cur_i + 2)
                                    )

                        for i in range(nt_pass):
                            if len(pend) == 2:
                                retire_one(i)
                            e = exp_pool.tile([P, 1024], md, tag="e", name="e")
                            kT_i = kTh[:, i * P : (i + 1) * P]
                            lo = max(i * P, c0p)
                            s = ps_pool.tile([P, 1024], f32, tag="s", name="s", bufs=3)
                            c = lo
                            first = True
                            while c < c1p:
                                ce = min((c // 512 + 1) * 512, c1p)
                                if first:
                                    nc.tensor.matmul(
                                        s[:, c - c0p : ce - c0p],
                                        lhsT=kT_i, rhs=qTh[:, c:ce],
                                        start=True, stop=True,
                                    )
                                    first = False
                                else:
                                    mm_noload(
                                        s[:, c - c0p : ce - c0p],
                                        kT_i, qTh[:, c:ce], True, True,
                                    )
                                c = ce
                            # proj/output unit between scores and the exp
                            # consumer keeps the PE streaming
                            if my_slots:
                                u = my_slots.pop(0)
                                if u:
                                    u()
                            nc.scalar.activation(
                                e[:, lo - c0p : 1024],
                                s[:, lo - c0p : 1024],
                                mybir.ActivationFunctionType.Exp,
                                scale=0.125,  # 1/sqrt(D)
                            )
                            if c0p <= i * P < c1p:
                                # SBUF->SBUF, so it can run on the otherwise
                                # idle gpsimd engine: keeps the exp->AV chain
                                # off the DVE drain queue
                                d0 = i * P - c0p
                                meng = nc.gpsimd if MASK_ENG == "pool" else nc.vector
                                meng.tensor_mul(
                                    e[:, d0 : d0 + P], e[:, d0 : d0 + P], mask
                                )
                            while carry and carry[0][1] <= i:
                                carry.pop(0)[0]()
                            pend.append((i, e))
                        while pend:
                            retire_one(nt_pass)
                        for u in my_slots:  # leftover units
                            if u:
                                u()
                for fn, _due in carry:
                    fn()

                if variant == "attn":
                    dump_debug(ctxn_sb)
                else:
                    for u in post_units:
                        u()

    nc.compile()
    return nc


def make_in_maps(x, pad_mask, Wq, Wk, Wv, Wo):
    """Host-side sharding: per-core input dict."""
    if MM_DTYPE == "bfloat16":
        import ml_dtypes

        in_np = ml_dtypes.bfloat16
    else:
        in_np = np.float16
    x = np.asarray(x, dtype=np.float32)
    pad_f = np.asarray(pad_mask).astype(np.float32).reshape(B, T, 1)
    Wq, Wk, Wv, Wo = (np.asarray(w, dtype=np.float32) for w in (Wq, Wk, Wv, Wo))
    in_maps = []
    for c in range(NCORES):
        b, g = c // GROUPS, c % GROUPS
        jr = slice(g * J, (g + 1) * J)
        in_maps.append(
            {
                "xT": np.ascontiguousarray(x[b].T).astype(in_np),
                "wq_t": np.ascontiguousarray(Wq[jr, :].T).astype(in_np),
                "wk_t": np.ascontiguousarray(Wk[jr, :].T).astype(in_np),
                "wv_t": np.ascontiguousarray(Wv[jr, :].T).astype(in_np),
                "wo_t": np.ascontiguousarray(Wo[:, jr].T).astype(in_np),
                "pad": np.ascontiguousarray(pad_f[b]),
            }
        )
    return in_maps


def unshard(results):
    """Sum the 4 tensor-parallel partials per batch; transpose back."""
    y = np.empty((B, T, C), dtype=np.float32)
    for b in range(B):
        acc = results[b * GROUPS]["yT"].astype(np.float32)
        for g in range(1, GROUPS):
            acc = acc + results[b * GROUPS + g]["yT"].astype(np.float32)
        y[b] = acc.T
    return y


def kernel(x, pad_mask, Wq, Wk, Wv, Wo):
    global _COMPILED
    from concourse.bass_utils import run_bass_kernel_spmd

    if _COMPILED is None:
        _COMPILED = build_program()
    in_maps = make_in_maps(x, pad_mask, Wq, Wk, Wv, Wo)
    res = run_bass_kernel_spmd(_COMPILED, in_maps, core_ids=list(range(NCORES)))
    return unshard(res.results)
